# revision 8
# baseline (speedup 1.0000x reference)
"""Trainium2 Bass kernel for nn_ChannelWiseMaxPoolWithCrossInfo.

Problem: x (8, 128, 64, 64) f32. 2x2 non-overlapping max-pool argmax per
channel c_pool gives, for each of the 1024 windows, the in-window position
(0..3) of the max. Output[b, c_pool, c_val, i] = the element of window i of
channel c_val at channel c_pool's argmax position. Shape (8, 128, 128, 1024).

Sharding: data-parallel over batch B=8 -> one batch element per NeuronCore.

Per-core algorithm (all shapes [partitions, free]):
  1. DMA x_b as X [128, 4096]; strided copies -> xw_j [128, 1024] (j = window
     position dh*2+dw), per-window max m, and bf16 equality masks
     cm_j = (xw_j == m) for j=0,1,2.
  2. Per pooled channel c: the output tile [c_val=128, window=1024] is a
     4-way select between xw_0..xw_3 keyed by channel c's argmax position.
     Broadcast mask rows cm_j[c, :] to all 128 partitions with a K=1 PE
     matmul (ones[1,128]^T @ row) into PSUM, then build the tile as
        out = xw_3; out = xw_2 where cm_2; out = xw_1 where cm_1;
        out = xw_0 where cm_0    (exact first-occurrence argmax semantics)
     using Act for the base copy and DVE copy_predicated (mask = PSUM fp32
     bitcast to int32, nonzero test) for the overwrites. DMA tile to HBM.
"""

import sys

sys.path.insert(0, "/opt/trn_rl_repo")

import numpy as np

import concourse.bacc as bacc
import concourse.mybir as mybir
import concourse.tile as tile
from concourse.alu_op_type import AluOpType
from concourse.bass_utils import run_bass_kernel_spmd

F32 = mybir.dt.float32
BF16 = mybir.dt.bfloat16
I32 = mybir.dt.int32

C = 128          # channels (both c_pool and c_val)
HW = 4096        # 64*64
P = 1024         # pooled positions (32*32)
HALF = 512
N_CORES = 8

_CACHE = {}


def _build_program():
    nc = bacc.Bacc("TRN2", target_bir_lowering=False)

    x_d = nc.dram_tensor("x", [C, HW], F32, kind="ExternalInput")
    # Row-selector stationary: wsel[k, c*128 + p] = 1.0 if k == c else 0.
    # lhsT = wsel[:, c*128:(c+1)*128] makes the PE broadcast mask row c
    # to all 128 output partitions: out[p, f] = cm[c, f].
    wsel_d = nc.dram_tensor("wsel", [C, C * C], BF16, kind="ExternalInput")
    out_d = nc.dram_tensor("out", [C, C, P], F32, kind="ExternalOutput")

    with tile.TileContext(nc) as tc:
        with (
            tc.tile_pool(name="persist", bufs=1) as pp,
            tc.tile_pool(name="outs", bufs=4) as op,
            tc.tile_pool(name="psum", bufs=2, space="PSUM") as psp,
        ):
            X = pp.tile([C, HW], F32)
            nc.sync.dma_start(out=X[:], in_=x_d[:])

            # X viewed [c, hp(32), dh(2), wp(32), dw(2)]
            X5 = X.rearrange("c (hp dh wp dw) -> c hp dh wp dw",
                             hp=32, dh=2, wp=32, dw=2)
            xw = []
            for j in range(4):
                dh, dw = j // 2, j % 2
                t = pp.tile([C, P], F32, name=f"xw{j}")
                tv = t.rearrange("c (hp wp) -> c hp wp", hp=32)
                nc.vector.tensor_copy(out=tv[:], in_=X5[:, :, dh, :, dw])
                xw.append(t)

            m = pp.tile([C, P], F32)
            t01 = pp.tile([C, P], F32)
            nc.vector.tensor_tensor(out=t01[:], in0=xw[0][:], in1=xw[1][:],
                                    op=AluOpType.max)
            nc.vector.tensor_tensor(out=m[:], in0=xw[2][:], in1=xw[3][:],
                                    op=AluOpType.max)
            nc.vector.tensor_tensor(out=m[:], in0=m[:], in1=t01[:],
                                    op=AluOpType.max)

            cm = []
            for j in range(3):
                t = pp.tile([C, P], BF16, name=f"cm{j}")
                nc.vector.tensor_tensor(out=t[:], in0=xw[j][:], in1=m[:],
                                        op=AluOpType.is_equal)
                cm.append(t)

            wsel = pp.tile([C, C * C], BF16)
            nc.sync.dma_start(out=wsel[:], in_=wsel_d[:])

            for c in range(C):
                ot = op.tile([C, P], F32, name="ot")
                nc.scalar.copy(ot[:], xw[3][:])
                wc = wsel[:, c * C:(c + 1) * C]
                for h in range(2):
                    ph = psp.tile([C, 3 * HALF], F32, name="mh")
                    sl = slice(h * HALF, (h + 1) * HALF)
                    for j in range(3):
                        nc.tensor.matmul(ph[:, j * HALF:(j + 1) * HALF],
                                         wc, cm[j][:, sl])
                    for j in (2, 1, 0):
                        nc.vector.copy_predicated(
                            out=ot[:, sl],
                            mask=ph[:, j * HALF:(j + 1) * HALF].bitcast(I32),
                            data=xw[j][:, sl])
                nc.sync.dma_start(out=out_d[c], in_=ot[:])

    nc.compile()
    return nc


def get_program():
    if "nc" not in _CACHE:
        _CACHE["nc"] = _build_program()
    return _CACHE["nc"]


def _make_wsel() -> np.ndarray:
    import ml_dtypes
    w = np.zeros((C, C, C), dtype=ml_dtypes.bfloat16)
    for k in range(C):
        w[k, k, :] = 1.0
    return w.reshape(C, C * C)


def kernel(x: np.ndarray) -> np.ndarray:
    assert x.shape == (N_CORES, C, 64, 64), x.shape
    x = np.ascontiguousarray(np.asarray(x, dtype=np.float32))
    nc = get_program()
    wsel = _make_wsel()
    in_maps = [{"x": x[b].reshape(C, HW), "wsel": wsel} for b in range(N_CORES)]
    res = run_bass_kernel_spmd(nc, in_maps, core_ids=list(range(N_CORES)))
    out = np.stack([res.results[b]["out"] for b in range(N_CORES)], axis=0)
    return out


# revision 9
# speedup vs baseline: 1.1478x; 1.1478x over previous
"""Trainium2 Bass kernel for nn_ChannelWiseMaxPoolWithCrossInfo.

Problem: x (8, 128, 64, 64) f32. 2x2 non-overlapping max-pool argmax per
channel c_pool gives, for each of the 1024 windows, the in-window position
(0..3) of the max. Output[b, c_pool, c_val, i] = the element of window i of
channel c_val at channel c_pool's argmax position. Shape (8, 128, 128, 1024).

Sharding: data-parallel over batch B=8 -> one batch element per NeuronCore.

Per-core algorithm (all shapes [partitions, free]):
  1. DMA x_b as X [128, 4096]; strided copies -> xw_j [128, 1024] (j = window
     position dh*2+dw), per-window max m, and bf16 equality masks
     cm_j = (xw_j == m) for j=0,1,2.
  2. Per pooled channel c: the output tile [c_val=128, window=1024] is a
     4-way select between xw_0..xw_3 keyed by channel c's argmax position.
     Broadcast mask rows cm_j[c, :] to all 128 partitions with a K=1 PE
     matmul (ones[1,128]^T @ row) into PSUM, then build the tile as
        out = xw_3; out = xw_2 where cm_2; out = xw_1 where cm_1;
        out = xw_0 where cm_0    (exact first-occurrence argmax semantics)
     using Act for the base copy and DVE copy_predicated (mask = PSUM fp32
     bitcast to int32, nonzero test) for the overwrites. DMA tile to HBM.
"""

import sys

sys.path.insert(0, "/opt/trn_rl_repo")

import numpy as np

import concourse.bacc as bacc
import concourse.mybir as mybir
import concourse.tile as tile
from concourse.alu_op_type import AluOpType
from concourse.bass_utils import run_bass_kernel_spmd

F32 = mybir.dt.float32
BF16 = mybir.dt.bfloat16
I32 = mybir.dt.int32

C = 128          # channels (both c_pool and c_val)
HW = 4096        # 64*64
P = 1024         # pooled positions (32*32)
HALF = 512
N_CORES = 8

_CACHE = {}


def _build_program():
    nc = bacc.Bacc("TRN2", target_bir_lowering=False)

    x_d = nc.dram_tensor("x", [C, HW], F32, kind="ExternalInput")
    # Row-selector stationary: wsel[k, c*128 + p] = 1.0 if k == c else 0.
    # lhsT = wsel[:, c*128:(c+1)*128] makes the PE broadcast mask row c
    # to all 128 output partitions: out[p, f] = cm[c, f].
    wsel_d = nc.dram_tensor("wsel", [C, C * C], BF16, kind="ExternalInput")
    out_d = nc.dram_tensor("out", [C, C, P], F32, kind="ExternalOutput")

    with tile.TileContext(nc) as tc:
        with (
            tc.tile_pool(name="persist", bufs=1) as pp,
            tc.tile_pool(name="outs", bufs=4) as op,
            tc.tile_pool(name="psum", bufs=2, space="PSUM") as psp,
        ):
            X = pp.tile([C, HW], F32)
            nc.sync.dma_start(out=X[:], in_=x_d[:])

            # X viewed [c, hp(32), dh(2), wp(32), dw(2)]
            X5 = X.rearrange("c (hp dh wp dw) -> c hp dh wp dw",
                             hp=32, dh=2, wp=32, dw=2)
            xw = []
            for j in range(4):
                dh, dw = j // 2, j % 2
                t = pp.tile([C, P], F32, name=f"xw{j}")
                tv = t.rearrange("c (hp wp) -> c hp wp", hp=32)
                nc.vector.tensor_copy(out=tv[:], in_=X5[:, :, dh, :, dw])
                xw.append(t)

            m = pp.tile([C, P], F32)
            t01 = pp.tile([C, P], F32)
            nc.vector.tensor_tensor(out=t01[:], in0=xw[0][:], in1=xw[1][:],
                                    op=AluOpType.max)
            nc.vector.tensor_tensor(out=m[:], in0=xw[2][:], in1=xw[3][:],
                                    op=AluOpType.max)
            nc.vector.tensor_tensor(out=m[:], in0=m[:], in1=t01[:],
                                    op=AluOpType.max)

            cm = []
            for j in range(3):
                t = pp.tile([C, P], BF16, name=f"cm{j}")
                nc.vector.tensor_tensor(out=t[:], in0=xw[j][:], in1=m[:],
                                        op=AluOpType.is_equal)
                cm.append(t)

            wsel = pp.tile([C, C * C], BF16)
            nc.sync.dma_start(out=wsel[:], in_=wsel_d[:])

            import os
            repeat = int(os.environ.get("KREPEAT", "1"))
            for c in range(C * repeat):
                c = c % C
                ot = op.tile([C, P], F32, name="ot")
                nc.scalar.copy(ot[:], xw[3][:])
                wc = wsel[:, c * C:(c + 1) * C]
                for h in range(2):
                    ph = psp.tile([C, 3 * HALF], F32, name="mh")
                    sl = slice(h * HALF, (h + 1) * HALF)
                    for j in range(3):
                        nc.tensor.matmul(ph[:, j * HALF:(j + 1) * HALF],
                                         wc, cm[j][:, sl])
                    for j in (2, 1, 0):
                        nc.vector.copy_predicated(
                            out=ot[:, sl],
                            mask=ph[:, j * HALF:(j + 1) * HALF].bitcast(I32),
                            data=xw[j][:, sl])
                nc.sync.dma_start(out=out_d[c], in_=ot[:])

    nc.compile()
    return nc


def get_program():
    if "nc" not in _CACHE:
        _CACHE["nc"] = _build_program()
    return _CACHE["nc"]


def _make_wsel() -> np.ndarray:
    import ml_dtypes
    w = np.zeros((C, C, C), dtype=ml_dtypes.bfloat16)
    for k in range(C):
        w[k, k, :] = 1.0
    return w.reshape(C, C * C)


def kernel(x: np.ndarray) -> np.ndarray:
    assert x.shape == (N_CORES, C, 64, 64), x.shape
    x = np.ascontiguousarray(np.asarray(x, dtype=np.float32))
    nc = get_program()
    wsel = _make_wsel()
    in_maps = [{"x": x[b].reshape(C, HW), "wsel": wsel} for b in range(N_CORES)]
    res = run_bass_kernel_spmd(nc, in_maps, core_ids=list(range(N_CORES)))
    out = np.stack([res.results[b]["out"] for b in range(N_CORES)], axis=0)
    return out
